# Initial kernel scaffold
#
"""CARAFE upsampling kernel for 8 Trainium2 NeuronCores.

Reference op (per batch b):
  xc   = conv1x1(x, w1) + b1                     # (CC=64, H, W)
  mask = conv3x3(xc, w2, pad=1) + b2             # (100, H, W)
  mask = softmax over the 25 kernel taps (per q in 4 = SF*SF groups)
  out[q, c, h, w] = sum_k mask[q, k, h, w] * x[c, h+di-2, w+dj-2]
  out pixel-shuffled by SF=2 -> (C, 2H, 2W)

Sharding: 8 shards = batch(4) x H-halves(2). Each core gets a padded
x slice [256, 36, 68] (2 halo rows / 2 zero-pad cols on each side) and
produces out rows [32 rows x 64 cols x 4 quadrants x 256 ch]; the host
performs the pixel shuffle + concat.
"""

import os
from functools import lru_cache

import numpy as np

import concourse.bass as bass
import concourse.mybir as mybir
from concourse import bacc
import concourse.tile as tile
from concourse.bass_utils import run_bass_kernel_spmd

F32 = mybir.dt.float32
BF16 = mybir.dt.bfloat16
import ml_dtypes as _mld

_BF16NP = _mld.bfloat16

# Problem constants (hardcoded; kernel.py must be self-contained).
B, C, H, W = 4, 256, 64, 64
CC = 64           # compressed channels
SF = 2            # scale factor
K5 = 5            # up-kernel
KA = K5 * K5      # 25 taps
NQ = SF * SF      # 4 quadrants
NM = NQ * KA      # 100 mask channels

HL = 32           # local (per-shard) output rows
HP = HL + 4       # padded rows
WP = W + 4        # padded cols
NPIX = HL * W     # 2048 output pixels per shard
NPADPIX = HP * WP # 2448 padded pixels

N_CORES = 8


def _build_program(trace_debug: bool = False):
    """Build the SPMD Bass program (identical on all cores)."""
    nc = bacc.Bacc("TRN2", target_bir_lowering=False, debug=False)

    # ---- DRAM parameters -------------------------------------------------
    x0_d = nc.dram_tensor("x0", [128, HP, WP], F32, kind="ExternalInput")
    x1_d = nc.dram_tensor("x1", [128, HP, WP], F32, kind="ExternalInput")
    w1t_d = nc.dram_tensor("w1t", [2, 128, CC], F32, kind="ExternalInput")
    w2t_d = nc.dram_tensor("w2t", [CC, 9, NM], F32, kind="ExternalInput")
    b1_d = nc.dram_tensor("b1v", [CC, 1], F32, kind="ExternalInput")
    b2_d = nc.dram_tensor("b2v", [NM, 1], F32, kind="ExternalInput")
    osum_d = nc.dram_tensor("osum", [NM, NQ], F32, kind="ExternalInput")
    orep_d = nc.dram_tensor("orep", [NQ, NM], F32, kind="ExternalInput")
    # gather selectors: sel4[k] is [NM, NQ] with column q = one-hot(q*25+k)
    sel4_d = nc.dram_tensor("sel4", [NM, KA, NQ], F32, kind="ExternalInput")
    # broadcast selectors: selb[q] is [NQ, 128] row-q of ones
    selb_d = nc.dram_tensor("selb", [NQ, NQ, 128], F32, kind="ExternalInput")
    # bf16 copies of the padded input, even- and odd-shifted (for DVE 2x mode
    # alignment: a window starting at odd dj reads the odd-shifted copy at an
    # even element offset)
    xbe_d = [nc.dram_tensor(f"xbe{c}", [128, HP, WP], BF16, kind="ExternalInput")
             for c in range(2)]
    xbo_d = [nc.dram_tensor(f"xbo{c}", [128, HP, WP], BF16, kind="ExternalInput")
             for c in range(2)]

    out_d = nc.dram_tensor("out", [2, 128, NQ, NPIX], F32, kind="ExternalOutput")
    msk_dbg_d = None
    if trace_debug:
        msk_dbg_d = nc.dram_tensor("msk_dbg", [NM, NPIX], F32, kind="ExternalOutput")

    with tile.TileContext(nc) as tc:
        with (
            tc.tile_pool(name="xpool", bufs=1) as xpool,
            tc.tile_pool(name="wpool", bufs=1) as wpool,
            tc.tile_pool(name="mpool", bufs=1) as mpool,
            tc.tile_pool(name="acc", bufs=1) as accpool,
            tc.tile_pool(name="scratch", bufs=2) as scratch,
            tc.tile_pool(name="psum", bufs=2, space="PSUM") as psum,
            tc.tile_pool(name="psum_rep", bufs=4, space="PSUM") as psum_rep,
        ):
            # ---- load inputs -------------------------------------------
            x0 = xpool.tile([128, HP, WP], F32)
            x1 = xpool.tile([128, HP, WP], F32)
            nc.sync.dma_start(x0[:], x0_d[:])
            nc.sync.dma_start(x1[:], x1_d[:])
            xbe0 = xpool.tile([128, HP, WP], BF16, tag="xbe0")
            xbe1 = xpool.tile([128, HP, WP], BF16, tag="xbe1")
            xbo0 = xpool.tile([128, HP, WP], BF16, tag="xbo0")
            xbo1 = xpool.tile([128, HP, WP], BF16, tag="xbo1")
            nc.sync.dma_start(xbe0[:], xbe_d[0][:])
            nc.sync.dma_start(xbe1[:], xbe_d[1][:])
            nc.sync.dma_start(xbo0[:], xbo_d[0][:])
            nc.sync.dma_start(xbo1[:], xbo_d[1][:])

            # partition dim must come first for SBUF: store as [128, 2, CC]
            w1sb = wpool.tile([128, 2, CC], F32, tag="w1sb")
            nc.sync.dma_start(w1sb[:, 0, :], w1t_d[0])
            nc.sync.dma_start(w1sb[:, 1, :], w1t_d[1])

            w2sb = wpool.tile([CC, 9, NM], F32, tag="w2sb")
            nc.sync.dma_start(w2sb[:], w2t_d[:])

            b1c = wpool.tile([CC, 1], F32, tag="b1c")
            nc.sync.dma_start(b1c[:], b1_d[:])
            b2c = wpool.tile([NM, 1], F32, tag="b2c")
            nc.sync.dma_start(b2c[:], b2_d[:])
            osum = wpool.tile([NM, NQ], F32, tag="osum")
            nc.sync.dma_start(osum[:], osum_d[:])
            orep = wpool.tile([NQ, NM], F32, tag="orep")
            nc.sync.dma_start(orep[:], orep_d[:])
            sel4 = wpool.tile([NM, KA, NQ], F32, tag="sel4")
            nc.sync.dma_start(sel4[:], sel4_d[:])
            selb = wpool.tile([NQ, NQ, 128], F32, tag="selb")
            nc.sync.dma_start(selb[:], selb_d[:])

            # ---- PE fences: make PE observe each input-DMA semaphore on a
            # tiny standalone matmul, so real (accumulating) matmuls don't
            # exceed the per-instruction sync-wait limit.
            for fap in (
                x0[:, 0, 0:1], x1[:, 0, 0:1], w1sb[:, 0, 0:1],
                w2sb[:, 0, 0:1], osum[:, 0:1], orep[:, 0:1],
                sel4[:, 0, 0:1], selb[:, 0, 0:1],
            ):
                psf = psum.tile([1, 1], F32, tag="psf")
                nc.tensor.matmul(psf[:], fap, fap, start=True, stop=True)

            # ---- stage A: conv1x1  xc[cc, pix'] over the padded grid ----
            xc = mpool.tile([CC, HP, WP], F32, tag="xc")
            xc_flat = xc[:].rearrange("c h w -> c (h w)")
            x0_flat = x0[:].rearrange("c h w -> c (h w)")
            x1_flat = x1[:].rearrange("c h w -> c (h w)")
            CHUNK = 512
            nchunks = (NPADPIX + CHUNK - 1) // CHUNK  # 5 (last = 400)
            for i in range(nchunks):
                n0 = i * CHUNK
                n1 = min(NPADPIX, n0 + CHUNK)
                ps = psum.tile([CC, CHUNK], F32, tag="ps")
                nc.tensor.matmul(
                    ps[:, : n1 - n0], w1sb[:, 0, :], x0_flat[:, n0:n1],
                    start=True, stop=False,
                )
                nc.tensor.matmul(
                    ps[:, : n1 - n0], w1sb[:, 1, :], x1_flat[:, n0:n1],
                    start=False, stop=True,
                )
                # += b1 while copying PSUM -> SBUF
                nc.vector.tensor_scalar_add(
                    xc_flat[:, n0:n1], ps[:, : n1 - n0], b1c[:, 0:1]
                )

            # ---- stage B: conv3x3 -> mask_raw, fused exp((.)+b2) -------
            # output pixels: h in 0..31 (padded row h+2), w in 0..63 (padded col w+2)
            msk_e = mpool.tile([NM, NPIX], F32, tag="msk_e")  # exp(mask_raw)
            HROWS = 8  # rows per 512-chunk
            for i in range(HL // HROWS):  # 4 chunks
                psm = psum.tile([NM, HROWS, W], F32, tag="ps")
                for tap in range(9):
                    dy, dx = tap // 3, tap % 3
                    rhs = xc[:, i * HROWS + 1 + dy : i * HROWS + 1 + dy + HROWS,
                             1 + dx : 1 + dx + W]
                    nc.tensor.matmul(
                        psm[:], w2sb[:, tap, :], rhs,
                        start=(tap == 0), stop=(tap == 8),
                    )
                me = msk_e[:].rearrange("m (h w) -> m h w", w=W)
                nc.scalar.activation(
                    me[:, i * HROWS : (i + 1) * HROWS, :], psm[:],
                    mybir.ActivationFunctionType.Exp, bias=b2c[:, 0:1],
                )

            # ---- stage C: softmax denominators + normalize -------------
            rs = mpool.tile([NQ, NPIX], F32, tag="rs")  # 1/sum per (q, pix)
            for i in range(NPIX // CHUNK):  # 4
                pss = psum.tile([NQ, CHUNK], F32, tag="ps")
                nc.tensor.matmul(
                    pss[:], osum[:], msk_e[:, i * CHUNK : (i + 1) * CHUNK],
                    start=True, stop=True,
                )
                nc.vector.reciprocal(rs[:, i * CHUNK : (i + 1) * CHUNK], pss[:])

            msk_n = mpool.tile([NM, NPIX], F32, tag="msk_n")
            for i in range(NPIX // CHUNK):
                psr = psum.tile([NM, CHUNK], F32, tag="ps")
                nc.tensor.matmul(
                    psr[:], orep[:], rs[:, i * CHUNK : (i + 1) * CHUNK],
                    start=True, stop=True,
                )
                nc.vector.tensor_mul(
                    msk_n[:, i * CHUNK : (i + 1) * CHUNK],
                    msk_e[:, i * CHUNK : (i + 1) * CHUNK], psr[:],
                )

            if trace_debug:
                nc.sync.dma_start(msk_dbg_d[:], msk_n[:])

            # ---- stage D1: combine (correctness-first) -----------------
            # acc[ch][c, q, pix] += msk_n[q*25+k, pix] * x[ch][c, window_k]
            acc0 = accpool.tile([128, NQ, NPIX], F32, tag="acc0")
            acc1 = accpool.tile([128, NQ, NPIX], F32, tag="acc1")
            nc.vector.memset(acc0[:], 0.0)
            nc.gpsimd.memset(acc1[:], 0.0)

            nadds = 0
            xbe = (xbe0, xbe1)
            xbo = (xbo0, xbo1)
            accs = (acc0, acc1)
            for k in range(KA):
                di, dj = k // 5, k % 5
                # pick the x copy whose window start is 4B-aligned in bf16
                xw, djw = (xbe, dj) if dj % 2 == 0 else (xbo, dj - 1)
                # stage 1: gather the 4 q-rows of tap k to partitions 0..3
                m4 = scratch.tile([NQ, NPIX], F32, tag="m4")
                for i in range(NPIX // CHUNK):
                    p4 = psum.tile([NQ, CHUNK], F32, tag="ps")
                    nc.tensor.matmul(
                        p4[:], sel4[:, k, :],
                        msk_n[:, i * CHUNK : (i + 1) * CHUNK],
                        start=True, stop=True,
                    )
                    nc.scalar.copy(m4[:, i * CHUNK : (i + 1) * CHUNK], p4[:])
                for q in range(NQ):
                    # stage 2: broadcast row q of m4 across 128 partitions
                    # (PE), cast to bf16 (ACT), multiply vs x-window (DVE
                    # 2x bf16), accumulate into fp32 acc (DVE/GPSIMD).
                    prod0 = scratch.tile([128, HL, W], BF16, tag="prod0")
                    prod1 = scratch.tile([128, HL, W], BF16, tag="prod1")
                    prods = [prod0, prod1]
                    prepb = scratch.tile([128, NPIX], BF16, tag="prepb")
                    for i in range(NPIX // CHUNK):
                        prep = psum_rep.tile([128, CHUNK], F32, tag="prep")
                        nc.tensor.matmul(
                            prep[:],
                            selb[:, q, :],
                            m4[:, i * CHUNK : (i + 1) * CHUNK],
                            start=True, stop=True,
                        )
                        nc.scalar.copy(
                            prepb[:, i * CHUNK : (i + 1) * CHUNK], prep[:]
                        )
                    prepv = prepb[:].rearrange("c (h w) -> c h w", w=W)
                    for ch in range(2):
                        xwin = xw[ch][:, di : di + HL, djw : djw + W]
                        nc.vector.tensor_mul(prods[ch][:], xwin, prepv)
                    for ch in range(2):
                        accv = accs[ch][:].rearrange("c q (h w) -> c q h w", w=W)
                        # split the adds between DVE and GPSIMD (~2:1)
                        eng = nc.gpsimd if (nadds % 2 == 0) else nc.vector
                        nadds += 1
                        eng.tensor_add(accv[:, q], accv[:, q], prods[ch][:])

            # ---- write out ---------------------------------------------
            nc.sync.dma_start(out_d[0], acc0[:])
            nc.sync.dma_start(out_d[1], acc1[:])

    nc.compile()
    return nc


@lru_cache(maxsize=2)
def _get_program(trace_debug: bool = False):
    return _build_program(trace_debug)


def _host_prep(x, w1, b1, w2, b2):
    """Build per-core input maps."""
    x = np.asarray(x, np.float32)
    w1 = np.asarray(w1, np.float32)
    b1 = np.asarray(b1, np.float32).reshape(CC, 1)
    w2 = np.asarray(w2, np.float32)
    b2 = np.asarray(b2, np.float32).reshape(NM, 1)

    w1t = np.ascontiguousarray(
        w1[:, :, 0, 0].T.reshape(2, 128, CC)
    )  # [c-tile, 128, CC]
    # w2: (100, 64, 3, 3) -> [cc, tap, m]
    w2t = np.ascontiguousarray(w2.transpose(1, 2, 3, 0).reshape(CC, 9, NM))
    osum = np.zeros((NM, NQ), np.float32)
    for q in range(NQ):
        osum[q * KA : (q + 1) * KA, q] = 1.0
    orep = np.ascontiguousarray(osum.T)
    sel4 = np.zeros((NM, KA, NQ), np.float32)
    for k in range(KA):
        for q in range(NQ):
            sel4[q * KA + k, k, q] = 1.0
    selb = np.zeros((NQ, NQ, 128), np.float32)
    for q in range(NQ):
        selb[q, q, :] = 1.0

    in_maps = []
    for s in range(N_CORES):
        b, hh = s // 2, s % 2
        h0 = hh * HL
        xpad = np.zeros((C, HP, WP), np.float32)
        r0 = max(0, h0 - 2)
        r1 = min(H, h0 + HL + 2)
        xpad[:, (r0 - h0 + 2) : (r1 - h0 + 2), 2 : 2 + W] = x[b, :, r0:r1, :]
        xb = xpad.astype(_BF16NP)
        xbo = np.zeros_like(xb)
        xbo[:, :, :-1] = xb[:, :, 1:]
        in_maps.append(
            {
                "x0": np.ascontiguousarray(xpad[:128]),
                "x1": np.ascontiguousarray(xpad[128:]),
                "xbe0": np.ascontiguousarray(xb[:128]),
                "xbe1": np.ascontiguousarray(xb[128:]),
                "xbo0": np.ascontiguousarray(xbo[:128]),
                "xbo1": np.ascontiguousarray(xbo[128:]),
                "w1t": w1t,
                "w2t": w2t,
                "b1v": b1,
                "b2v": b2,
                "osum": osum,
                "orep": orep,
                "sel4": sel4,
                "selb": selb,
            }
        )
    return in_maps


def _host_post(results):
    """Reassemble full output from per-core results."""
    out = np.empty((B, C, H * SF, W * SF), np.float32)
    for s in range(N_CORES):
        b, hh = s // 2, s % 2
        o = results[s]["out"]  # [2, 128, NQ, NPIX]
        o = o.reshape(2, 128, NQ, HL, W).reshape(C, SF, SF, HL, W)
        # out[c, 2h+sh, 2w+sw] = o[c, sh, sw, h, w]
        o = o.transpose(0, 3, 1, 4, 2).reshape(C, HL * SF, W * SF)
        out[b, :, hh * HL * SF : (hh + 1) * HL * SF, :] = o
    return out


def kernel(x, w1, b1, w2, b2):
    nc = _get_program(bool(int(os.environ.get("CARAFE_DEBUG", "0"))))
    in_maps = _host_prep(x, w1, b1, w2, b2)
    res = run_bass_kernel_spmd(nc, in_maps, list(range(N_CORES)))
    return _host_post(res.results)



# revision 11
# speedup vs baseline: 1.5867x; 1.5867x over previous
"""CARAFE upsampling kernel for 8 Trainium2 NeuronCores — banded-GEMM version.

Reference op (per batch b):
  xc   = conv1x1(x, w1) + b1                     # (CC=64, H, W)
  mask = conv3x3(xc, w2, pad=1) + b2             # (100, H, W)
  mask = softmax over the 25 kernel taps (per q in 4 = SF*SF groups)
  out[q, c, h, w] = sum_k mask[q, k, h, w] * x[c, h+di-2, w+dj-2]
  out pixel-shuffled by SF=2 -> (C, 2H, 2W)

Sharding: 8 shards = batch(4) x H-halves(2), 32 output rows each.

Combine strategy: for each output row h and q-pair P, the 25-tap weighted
gather is a matmul contracting over the padded-w axis (68):
    psum[(qp,w), c] += sum_di sum_w' Band_{h,di,P}[w', (qp,w)] * xT[w', h+di, c]
where Band[w+dj, (qp,w)] = mask_n[q*25+di*5+dj, h, w] is a banded matrix
built from the normalized mask by a diagonal-scatter DMA, and
xT[wpad, hpad, c] is a host-provided transpose of the padded input.
All matmul operands bf16; PSUM accumulates fp32; output stored bf16.
"""

import os
from functools import lru_cache

import numpy as np
import ml_dtypes

import concourse.mybir as mybir
from concourse import bacc
import concourse.tile as tile
from concourse.bass import AP
from concourse.bass_utils import run_bass_kernel_spmd

F32 = mybir.dt.float32
BF16 = mybir.dt.bfloat16
_BF16NP = ml_dtypes.bfloat16
AF = mybir.ActivationFunctionType

# Problem constants (hardcoded; kernel.py must be self-contained).
B, C, H, W = 4, 256, 64, 64
CC = 64           # compressed channels
SF = 2            # scale factor
KA = 25           # taps
NQ = 4            # quadrants
NM = NQ * KA      # 100 mask channels

HL = 32           # local (per-shard) output rows
HP = HL + 4       # padded rows (2 halo each side)
WP2 = W + 4       # padded cols
NPIX = HL * W     # 2048 output pixels per shard
NPAD = HP * WP2   # 2448 padded pixels

BFREE = 5 * 2 * 2 * W * HL   # band free size = 40960
QHW = W * HL                 # 2048 (per-(di,q) block in band cols)

N_CORES = 8


def _scatter_band(nc, msk_T, stg_d):
    """Scatter msk_T[100, (w*32+h)] into the DRAM staging band image.

    stg[w+dj, di, P, qp, w, h] = msk_T[q*25+di*5+dj, w*32+h], q = 2P+qp.
    SBUF APs cannot express diagonals (partition stride must be a whole
    row multiple), but DRAM APs are flat - so the diagonal lives on the
    DRAM side.  One DMA per (di, q); dims (dj, w-diag, h).  All stg DMAs
    go on the qAct queue (nc.scalar) so zero-fill -> scatter -> band-in
    are ordered by the queue FIFO.
    """
    mt = msk_T[:].tensor
    st = stg_d[:].tensor
    for di in range(5):
        for q in range(4):
            src = AP(mt, (q * 25 + di * 5) * NPIX,
                     [[NPIX, 5], [HL, W], [1, HL]])
            dst = AP(st, di * (2 * 2 * QHW) + q * QHW,
                     [[BFREE, 5], [BFREE + HL, W], [1, HL]])
            nc.scalar.dma_start(dst, src)


def _build_program():
    nc = bacc.Bacc("TRN2", target_bir_lowering=False, debug=False)

    # ---- DRAM parameters -------------------------------------------------
    xcm0_d = nc.dram_tensor("xcm0", [128, NPAD], BF16, kind="ExternalInput")
    xcm1_d = nc.dram_tensor("xcm1", [128, NPAD], BF16, kind="ExternalInput")
    xt_d = nc.dram_tensor("xt", [WP2, HP, C], BF16, kind="ExternalInput")
    w1t_d = nc.dram_tensor("w1t", [2, 128, CC], BF16, kind="ExternalInput")
    w2t_d = nc.dram_tensor("w2t", [CC, 9, NM], BF16, kind="ExternalInput")
    b1_d = nc.dram_tensor("b1v", [CC, 1], F32, kind="ExternalInput")
    b2_d = nc.dram_tensor("b2v", [NM, 1], F32, kind="ExternalInput")
    osum_d = nc.dram_tensor("osum", [NM, NQ], BF16, kind="ExternalInput")
    orep_d = nc.dram_tensor("orep", [NQ, NM], BF16, kind="ExternalInput")
    out_d = nc.dram_tensor("out", [128, 2, HL, C], BF16, kind="ExternalOutput")
    stg_d = nc.dram_tensor("stg", [WP2, BFREE], BF16, kind="Internal")

    with tile.TileContext(nc) as tc:
        with (
            tc.tile_pool(name="wpool", bufs=1) as wpool,
            tc.tile_pool(name="xpool", bufs=1) as xpool,
            tc.tile_pool(name="mpool", bufs=1) as mpool,
            tc.tile_pool(name="bandp", bufs=1) as bandp,
            tc.tile_pool(name="opool", bufs=1) as opool,
            tc.tile_pool(name="psA", bufs=2, space="PSUM") as psA,
            tc.tile_pool(name="psB", bufs=2, space="PSUM") as psB,
            tc.tile_pool(name="psO", bufs=4, space="PSUM") as psO,
        ):
            # ---- load inputs -------------------------------------------
            w1sb = wpool.tile([128, 2, CC], BF16, tag="w1sb")
            nc.sync.dma_start(w1sb[:, 0, :], w1t_d[0])
            nc.sync.dma_start(w1sb[:, 1, :], w1t_d[1])
            w2sb = wpool.tile([CC, 9, NM], BF16, tag="w2sb")
            nc.sync.dma_start(w2sb[:], w2t_d[:])
            b1c = wpool.tile([CC, 1], F32, tag="b1c")
            nc.sync.dma_start(b1c[:], b1_d[:])
            b2c = wpool.tile([NM, 1], F32, tag="b2c")
            nc.sync.dma_start(b2c[:], b2_d[:])
            osum = wpool.tile([NM, NQ], BF16, tag="osum")
            nc.sync.dma_start(osum[:], osum_d[:])
            orep = wpool.tile([NQ, NM], BF16, tag="orep")
            nc.sync.dma_start(orep[:], orep_d[:])

            xcm0 = xpool.tile([128, NPAD], BF16, tag="xcm0")
            xcm1 = xpool.tile([128, NPAD], BF16, tag="xcm1")
            nc.sync.dma_start(xcm0[:], xcm0_d[:])
            nc.sync.dma_start(xcm1[:], xcm1_d[:])
            xt = xpool.tile([WP2, HP, C], BF16, tag="xt")
            nc.sync.dma_start(xt[:], xt_d[:])

            band = bandp.tile([WP2, 5, 2, 2, W, HL], BF16, tag="band")

            # Zero-fill the DRAM staging image (structural zeros of the
            # band).  Off the critical path: runs on qAct during convs.
            zt = xpool.tile([128, 4096], BF16, tag="zt")
            nc.gpsimd.memset(zt[:], 0.0)
            NZT = 128 * 4096
            NSTG = WP2 * BFREE  # 2785280 = 5*NZT + 40*4096
            zoff = 0
            while zoff < NSTG:
                n = min(NZT, NSTG - zoff)
                rows = n // 4096
                dst = AP(stg_d[:].tensor, zoff, [[4096, rows], [1, 4096]])
                nc.scalar.dma_start(dst, zt[0:rows, :])
                zoff += n

            # ---- PE fences on DMA'd matmul operands --------------------
            for fap in (
                w1sb[:, 0, 0:1], w2sb[:, 0, 0:1], osum[:, 0:1],
                orep[:, 0:1], xcm0[:, 0:1], xcm1[:, 0:1], xt[:, 0, 0:1],
            ):
                psf = psA.tile([1, 1], F32, tag="psa")
                nc.tensor.matmul(psf[:], fap, fap, start=True, stop=True)

            # ---- stage A: conv1x1 over the padded grid -> xcb bf16 -----
            xcb = mpool.tile([CC, NPAD], BF16, tag="xcb")
            CHUNK = 512
            nchunks = (NPAD + CHUNK - 1) // CHUNK  # 5 (last = 400)
            for i in range(nchunks):
                n0 = i * CHUNK
                n1 = min(NPAD, n0 + CHUNK)
                ps = psA.tile([CC, CHUNK], F32, tag="psa")
                nc.tensor.matmul(ps[:, : n1 - n0], w1sb[:, 0, :],
                                 xcm0[:, n0:n1], start=True, stop=False)
                nc.tensor.matmul(ps[:, : n1 - n0], w1sb[:, 1, :],
                                 xcm1[:, n0:n1], start=False, stop=True)
                nc.vector.tensor_scalar_add(xcb[:, n0:n1], ps[:, : n1 - n0],
                                            b1c[:, 0:1])

            xcb3 = xcb[:].rearrange("c (h w) -> c h w", w=WP2)

            # ---- stage B: conv3x3 -> exp(mask+b2), bf16 ----------------
            msk_e = mpool.tile([NM, HL, W], BF16, tag="msk_e")
            HR = 8
            for i in range(HL // HR):  # 4 chunks of 8 rows
                psm = psB.tile([NM, HR, W], F32, tag="psb")
                for tap in range(9):
                    dy, dx = tap // 3, tap % 3
                    rhs = xcb3[:, i * HR + 1 + dy: i * HR + 1 + dy + HR,
                               1 + dx: 1 + dx + W]
                    nc.tensor.matmul(psm[:], w2sb[:, tap, :], rhs,
                                     start=(tap == 0), stop=(tap == 8))
                nc.scalar.activation(msk_e[:, i * HR:(i + 1) * HR, :], psm[:],
                                     AF.Exp, bias=b2c[:, 0:1])

            msk_ef = msk_e[:].rearrange("m h w -> m (h w)")

            # ---- stage C: softmax denominators -> rs = 1/sum, bf16 -----
            # 1/S = exp(-ln(S)); Ln and Exp share one ACT table set
            # (natural_log_exp_and_others), and ACT Reciprocal is banned.
            rs = mpool.tile([NQ, NPIX], BF16, tag="rs")
            tln = mpool.tile([NQ, NPIX], F32, tag="tln")
            for i in range(NPIX // CHUNK):
                pss = psA.tile([NQ, CHUNK], F32, tag="psa")
                nc.tensor.matmul(pss[:], osum[:],
                                 msk_ef[:, i * CHUNK:(i + 1) * CHUNK],
                                 start=True, stop=True)
                nc.scalar.activation(tln[:, i * CHUNK:(i + 1) * CHUNK],
                                     pss[:], AF.Ln)
                nc.scalar.activation(rs[:, i * CHUNK:(i + 1) * CHUNK],
                                     tln[:, i * CHUNK:(i + 1) * CHUNK],
                                     AF.Exp, scale=-1.0)

            # ---- stage D: normalize, TRANSPOSED write  msk_T[m, w, h] --
            msk_T = mpool.tile([NM, W, HL], BF16, tag="msk_T")
            for i in range(HL // HR):
                psr = psB.tile([NM, CHUNK], F32, tag="psb")
                nc.tensor.matmul(psr[:], orep[:],
                                 rs[:, i * CHUNK:(i + 1) * CHUNK],
                                 start=True, stop=True)
                # out iterated in (h, w) order, written at col w*HL + h
                outap = msk_T[:, :, i * HR:(i + 1) * HR].rearrange(
                    "m w h -> m h w")
                nc.vector.tensor_mul(outap, msk_e[:, i * HR:(i + 1) * HR, :],
                                     psr[:].rearrange("m (h w) -> m h w", w=W))

            # ---- stage E: scatter msk_T -> stg (DRAM), then band-in ----
            _scatter_band(nc, msk_T, stg_d)
            # band-in: identity copy per di-chunk (pipelines with stage F)
            for di in range(5):
                src = AP(stg_d[:].tensor, di * (2 * 2 * QHW),
                         [[BFREE, WP2], [1, 2 * 2 * QHW]])
                nc.scalar.dma_start(band[:, di], src)

            # ---- stage F: banded matmuls + copy-out --------------------
            obuf = opool.tile([128, 2, HL, C], BF16, tag="obuf")
            HS = 4  # h-stripe
            ncopy = 0
            for s in range(HL // HS):
                psos = [psO.tile([128, 2, C], F32, tag="pso", name=f"pso{s}_{j}")
                        for j in range(HS)]
                for di in range(5):
                    for hh in range(HS):
                        h = s * HS + hh
                        for P in range(2):
                            # start=True clears has_written bits for the
                            # WHOLE bank, so only the very first matmul
                            # into this tile may set it; the P=1 group
                            # then starts via cleared bits (overwrite).
                            nc.tensor.matmul(
                                psos[hh][:, P, :],
                                band[:, di, P, :, :, h],
                                xt[:, h + di, :],
                                start=(di == 0 and P == 0), stop=(di == 4),
                            )
                for hh in range(HS):
                    h = s * HS + hh
                    if ncopy % 2 == 0:
                        nc.vector.tensor_copy(obuf[:, :, h, :], psos[hh][:])
                    else:
                        nc.scalar.copy(obuf[:, :, h, :], psos[hh][:])
                    ncopy += 1

            # ---- write out ---------------------------------------------
            for i in range(4):
                h0, h1 = i * 8, (i + 1) * 8
                nc.sync.dma_start(out_d[:, :, h0:h1, :], obuf[:, :, h0:h1, :])

    nc.compile()
    return nc


@lru_cache(maxsize=1)
def _get_program(trace_debug: bool = False):
    return _build_program()


def _host_prep(x, w1, b1, w2, b2):
    """Build per-core input maps."""
    x = np.asarray(x, np.float32)
    w1 = np.asarray(w1, np.float32)
    b1 = np.asarray(b1, np.float32).reshape(CC, 1)
    w2 = np.asarray(w2, np.float32)
    b2 = np.asarray(b2, np.float32).reshape(NM, 1)

    w1t = np.ascontiguousarray(
        w1[:, :, 0, 0].T.reshape(2, 128, CC)).astype(_BF16NP)
    w2t = np.ascontiguousarray(
        w2.transpose(1, 2, 3, 0).reshape(CC, 9, NM)).astype(_BF16NP)
    osum = np.zeros((NM, NQ), np.float32)
    for q in range(NQ):
        osum[q * KA:(q + 1) * KA, q] = 1.0
    orep = np.ascontiguousarray(osum.T).astype(_BF16NP)
    osum = osum.astype(_BF16NP)

    in_maps = []
    for s in range(N_CORES):
        b, hh = s // 2, s % 2
        h0 = hh * HL
        xpad = np.zeros((C, HP, WP2), np.float32)
        r0 = max(0, h0 - 2)
        r1 = min(H, h0 + HL + 2)
        xpad[:, (r0 - h0 + 2):(r1 - h0 + 2), 2:2 + W] = x[b, :, r0:r1, :]
        xb = xpad.astype(_BF16NP)
        in_maps.append({
            "xcm0": np.ascontiguousarray(xb[:128].reshape(128, NPAD)),
            "xcm1": np.ascontiguousarray(xb[128:].reshape(128, NPAD)),
            "xt": np.ascontiguousarray(xb.transpose(2, 1, 0)),
            "w1t": w1t,
            "w2t": w2t,
            "b1v": b1,
            "b2v": b2,
            "osum": osum,
            "orep": orep,
        })
    return in_maps


def _host_post(results):
    """Reassemble full output from per-core results."""
    out = np.empty((B, C, H * SF, W * SF), np.float32)
    for s in range(N_CORES):
        b, hh = s // 2, s % 2
        o = results[s]["out"].astype(np.float32)  # [128(qp,w), 2(P), 32(h), 256(c)]
        o = o.reshape(2, W, 2, HL, C)             # [qp, w, P, h, c]
        o = o.transpose(4, 3, 2, 1, 0).reshape(C, HL * SF, W * SF)
        out[b, :, hh * HL * SF:(hh + 1) * HL * SF, :] = o
    return out


def kernel(x, w1, b1, w2, b2):
    nc = _get_program()
    in_maps = _host_prep(x, w1, b1, w2, b2)
    res = run_bass_kernel_spmd(nc, in_maps, list(range(N_CORES)))
    return _host_post(res.results)
